# revision 12
# baseline (speedup 1.0000x reference)
"""Trainium2 Bass kernel for a small BertLayer (B=64, S=512, HID=40, H=4, INTER=20).

Strategy: data-parallel over batch across 8 NeuronCores (8 sequences/core).
Everything for a core's shard lives in SBUF; HBM traffic is just in+out.

Per-core dataflow (transposed-activation layout, hid-on-partitions for matmuls):
  LN1 (stats on DVE, rsqrt via ACT ln/exp)  ->  hhat1 (bf16, tokens-on-partitions)
  PE-transpose hhat1 -> hhat1T [41,512] (ones row appended for bias folding)
  qT/kT = WqA/WkA @ hhat1T   (PSUM fp32 -> SBUF fp32, used as f32r in scores)
  v_aug = hhat1T.T @ WvA     (natural [512,44]: per-head [v_h | ones]; bf16)
  per head: scoresT[k,q] = kT_h.T @ qT_h (f32r); exp on ACT (scale 1/sqrt(DH),
            [128,1024] chunks, bf16 out); ctx[q, 11h:11h+11] += pT.T @ v_aug
            (denominator comes out in column 11h+10 via the ones column)
  normalize ctx by reciprocal(denoms) (one DVE tensor_tensor w/ free-dim bcast)
  PE-transpose ctxn -> ctxnT [41,512]; attn = ctxnT.T @ WoA + x (fp32 residual)
  LN2 -> hhat2T; interT = W1A @ hhat2T; spill interT to SBUF (all batches);
  one Gelu over [20, 8*512]; out = geluT.T @ W2A + attn; DMA out.
"""

import os
import sys
import math
import numpy as np

sys.path.insert(0, "/opt/trn_rl_repo")

import concourse.bass as bass
import concourse.bacc as bacc
import concourse.tile as tile
import concourse.mybir as mybir
from concourse.masks import make_identity

FP32 = mybir.dt.float32
F32R = mybir.dt.float32r
BF16 = mybir.dt.bfloat16

B, S, HID, H, DH, INTER = 64, 512, 40, 4, 10, 20
NCORES = 8
BLOC = B // NCORES          # 8 sequences per core
NCH = S // 128              # 4 token chunks of 128
EPS = 1e-5
ISQD = 1.0 / math.sqrt(DH)

PARAMS = [
    ("ln1_g", (HID,)), ("ln1_b", (HID,)),
    ("Wq", (HID, HID)), ("bq", (HID,)),
    ("Wk", (HID, HID)), ("bk", (HID,)),
    ("Wv", (HID, HID)), ("bv", (HID,)),
    ("Wo", (HID, HID)), ("bo", (HID,)),
    ("ln2_g", (HID,)), ("ln2_b", (HID,)),
    ("W1", (HID, INTER)), ("b1", (INTER,)),
    ("W2", (INTER, HID)), ("b2", (HID,)),
]


def _f32r(ap):
    return ap.bitcast(F32R)


def build_bert_kernel(nc, tc, ins, out_dram, bloc=BLOC):
    """Emit the per-core kernel into TileContext tc. ins: dict name->AP."""
    from contextlib import ExitStack

    ctx = ExitStack()
    x_dram = ins["hidden_states"]  # [bloc, 512, 40]

    # ---------------- persistent SBUF state ----------------
    def sb(name, shape, dtype=FP32):
        return nc.alloc_sbuf_tensor(name, list(shape), dtype).ap()

    x_all = sb("x_all", [128, bloc, NCH, HID])           # fp32 input, resident
    attn_all = sb("attn_all", [128, bloc, NCH, HID])     # fp32 attn residual
    mv1_all = sb("mv1_all", [128, bloc, NCH, 2])         # LN1 (mean, var)
    rsig1_all = sb("rsig1_all", [128, bloc, NCH])        # LN1 rsqrt(var+eps)
    interT_all = sb("interT_all", [INTER, bloc, S])      # mm1 out, fp32
    geluT_all = sb("geluT_all", [INTER + 1, bloc, S], BF16)  # gelu out + ones row
    ident_bf = sb("ident_bf", [128, 128], BF16)

    # double-buffered transposed activations (ones row preset once)
    h1augT = [sb(f"h1augT{i}", [HID + 1, S], BF16) for i in range(2)]
    ctxnaugT = [sb(f"ctxnaugT{i}", [HID + 1, S], BF16) for i in range(2)]
    h2augT = [sb(f"h2augT{i}", [HID + 1, S], BF16) for i in range(2)]
    qT_sb = [sb(f"qT_sb{i}", [128, S], F32R) for i in range(2)]
    kT_sb = [sb(f"kT_sb{i}", [128, S], F32R) for i in range(2)]

    # augmented weights
    WqA = sb("WqA", [HID + 1, 128], BF16)
    WkA = sb("WkA", [HID + 1, 128], BF16)
    WvA = sb("WvA", [HID + 1, H * (DH + 1)], BF16)
    WoA = sb("WoA", [HID + 1, HID], BF16)
    W1A = sb("W1A", [HID + 1, INTER], BF16)
    W2A = sb("W2A", [INTER + 1, HID], BF16)

    # ---------------- pools ----------------
    pool = ctx.enter_context(tc.tile_pool(name="work", bufs=3))
    pool2 = ctx.enter_context(tc.tile_pool(name="work2", bufs=2))
    psA = ctx.enter_context(tc.tile_pool(name="psA", bufs=2, space="PSUM"))
    psB = ctx.enter_context(tc.tile_pool(name="psB", bufs=3, space="PSUM"))
    psC = ctx.enter_context(tc.tile_pool(name="psC", bufs=1, space="PSUM"))

    # ---------------- constants & weight prep ----------------
    const0 = sb("const0", [128, 1])
    nc.vector.memset(const0, 0.0)
    nc.const_aps.aps[(FP32, 0.0)] = const0
    constEps = sb("constEps", [128, 1])
    nc.vector.memset(constEps, EPS)
    nc.const_aps.aps[(FP32, EPS)] = constEps

    make_identity(nc, ident_bf)

    # ones rows live at partitions 40/20, which compute engines cannot
    # address (start partition must be 0/32/64/96) -> write them via DMA
    # from a partition-0 staging row.
    ones_bf = sb("ones_bf", [1, bloc * S], BF16)
    nc.vector.memset(ones_bf, 1.0)
    for t in (h1augT, ctxnaugT, h2augT):
        for i in range(2):
            nc.sync.dma_start(out=t[i][HID : HID + 1, :], in_=ones_bf[:, 0:S])
    nc.sync.dma_start(
        out=geluT_all[INTER : INTER + 1, :, :],
        in_=ones_bf.rearrange("one (b s) -> one b s", b=bloc),
    )

    # load raw params
    P = {}
    for name, shape in PARAMS:
        if name == "hidden_states":
            continue
        ap = ins[name]
        if len(shape) == 1:
            if name in ("ln1_g", "ln1_b", "ln2_g", "ln2_b"):
                t = sb(f"p_{name}", [shape[0], 1])
                nc.sync.dma_start(out=t, in_=ap.rearrange("(p one) -> p one", one=1))
            else:
                t = sb(f"p_{name}", [1, shape[0]])
                nc.sync.dma_start(out=t, in_=ap.rearrange("(one m) -> one m", one=1))
        else:
            t = sb(f"p_{name}", list(shape))
            nc.sync.dma_start(out=t, in_=ap)
        P[name] = t

    # Augmented-weight assembly. Body rows ([0:40]) are built/cast at
    # partition base 0 by DVE; the bias row is built in a [1, N] staging
    # row (partition 0), cast to bf16, and DMA'd into row HID/INTER.
    scr_body = sb("wprep_body", [HID, 128])
    scr_row = sb("wprep_row", [1, 128])
    scr_row_bf = sb("wprep_row_bf", [1, 128], BF16)

    def head_view(ap_t, base_off, stride):
        return bass.AP(
            ap_t.tensor, ap_t.offset + base_off, [ap_t.ap[0], [stride, H], [1, DH]]
        )

    def build_aug(dst, W_sb, gain_col, lnb_col, b_row, n_cols, head_stride=None):
        """dst[0:rows] = gain*W (optionally scattered per-head), dst[rows] =
        lnb@W + b, via staging rows."""
        rows = W_sb.shape[0]
        nc.vector.memset(scr_body[:, 0:n_cols], 0.0)
        nc.vector.memset(scr_row[:, 0:n_cols], 0.0)
        if head_stride is None:
            if gain_col is None:
                nc.vector.tensor_copy(out=scr_body[0:rows, 0:n_cols], in_=W_sb)
            else:
                nc.vector.tensor_scalar_mul(
                    out=scr_body[0:rows, 0:n_cols], in0=W_sb, scalar1=gain_col
                )
        else:
            dst_v = head_view(scr_body[0:rows, 0:128], 0, head_stride)
            src_v = head_view(W_sb, 0, DH)
            if gain_col is None:
                nc.vector.tensor_copy(out=dst_v, in_=src_v)
            else:
                nc.vector.tensor_scalar_mul(out=dst_v, in0=src_v, scalar1=gain_col)
        if lnb_col is not None:
            bp = psB.tile([1, W_sb.shape[1]], FP32, tag="ps")
            nc.tensor.matmul(bp, lhsT=lnb_col, rhs=W_sb, start=True, stop=True)
            if head_stride is None:
                nc.vector.tensor_tensor(
                    out=scr_row[:, 0 : W_sb.shape[1]], in0=bp, in1=b_row,
                    op=mybir.AluOpType.add,
                )
            else:
                nc.vector.tensor_tensor(
                    out=head_view(scr_row, 0, head_stride),
                    in0=head_view(bp, 0, DH),
                    in1=head_view(b_row, 0, DH),
                    op=mybir.AluOpType.add,
                )
        else:
            if head_stride is None:
                nc.vector.tensor_copy(out=scr_row[:, 0 : b_row.shape[-1]], in_=b_row)
            else:
                nc.vector.tensor_copy(
                    out=head_view(scr_row, 0, head_stride), in_=head_view(b_row, 0, DH)
                )
        nc.vector.tensor_copy(out=dst[0:rows, :], in_=scr_body[0:rows, 0:n_cols])
        nc.vector.tensor_copy(out=scr_row_bf[:, 0:n_cols], in_=scr_row[:, 0:n_cols])
        nc.sync.dma_start(out=dst[rows : rows + 1, :], in_=scr_row_bf[:, 0:n_cols])

    build_aug(WqA, P["Wq"], P["ln1_g"], P["ln1_b"], P["bq"], 128, head_stride=32)
    build_aug(WkA, P["Wk"], P["ln1_g"], P["ln1_b"], P["bk"], 128, head_stride=32)
    build_aug(
        WvA, P["Wv"], P["ln1_g"], P["ln1_b"], P["bv"], H * (DH + 1),
        head_stride=DH + 1,
    )
    # ones column marker in WvA bias row: col 11h+10 = 1.0 (free offset only,
    # partition 0 -> DVE-legal); redo the bf16 cast+DMA for the row.
    ones_c = bass.AP(scr_row.tensor, scr_row.offset + DH, [scr_row.ap[0], [DH + 1, H]])
    nc.vector.memset(ones_c, 1.0)
    nc.vector.tensor_copy(
        out=scr_row_bf[:, 0 : H * (DH + 1)], in_=scr_row[:, 0 : H * (DH + 1)]
    )
    nc.sync.dma_start(
        out=WvA[HID : HID + 1, :], in_=scr_row_bf[:, 0 : H * (DH + 1)]
    )
    build_aug(WoA, P["Wo"], None, None, P["bo"], HID)
    build_aug(W1A, P["W1"], P["ln2_g"], P["ln2_b"], P["b1"], INTER)
    build_aug(W2A, P["W2"], None, None, P["b2"], HID)

    # ---------------- phase 1: load x, LN1 stats ----------------
    for b in range(bloc):
        nc.sync.dma_start(
            out=x_all[:, b, :, :],
            in_=x_dram[b].rearrange("(c p) d -> p c d", p=128),
        )
        bn6 = pool.tile([128, NCH, 6], FP32, tag="bn6")
        for c in range(NCH):
            nc.vector.bn_stats(out=bn6[:, c, :], in_=x_all[:, b, c, :])
            nc.vector.bn_aggr(out=mv1_all[:, b, c, :], in_=bn6[:, c, :])

    # rsig1 for all batches in two ACT instrs (ln + exp; same table set as Exp)
    var1 = bass.AP(
        mv1_all.tensor, mv1_all.offset + 1, [[bloc * NCH * 2, 128], [NCH * 2, bloc], [2, NCH]]
    )
    lnv1 = pool.tile([128, bloc, NCH], FP32, tag="lnv")
    nc.scalar.activation(out=lnv1, in_=var1, func=mybir.ActivationFunctionType.Ln, bias=EPS)
    nc.scalar.activation(
        out=rsig1_all[:, :, :], in_=lnv1, func=mybir.ActivationFunctionType.Exp, scale=-0.5
    )

    # ---------------- phase 2: attention + FFN front, per batch ----------------
    for b in range(bloc):
        d = b % 2
        # hhat1 (bf16) and transpose
        h1n = pool.tile([128, NCH, HID], BF16, tag="h1n")
        for c in range(NCH):
            nc.vector.tensor_scalar(
                out=h1n[:, c, :], in0=x_all[:, b, c, :],
                scalar1=mv1_all[:, b, c, 0:1], scalar2=rsig1_all[:, b, c : c + 1],
                op0=mybir.AluOpType.subtract, op1=mybir.AluOpType.mult,
            )
        tr1 = psB.tile([HID, S], BF16, tag="ps")
        for c in range(NCH):
            nc.tensor.matmul(
                tr1[:, 128 * c : 128 * (c + 1)], lhsT=h1n[:, c, :], rhs=ident_bf,
                is_transpose=True, start=(c == 0), stop=(c == NCH - 1),
            )
        nc.vector.tensor_copy(out=h1augT[d][0:HID, :], in_=tr1)

        # q/k projections (transposed out) + v (natural, head-augmented)
        qTp = psB.tile([128, S], FP32, tag="ps")
        nc.tensor.matmul(qTp, lhsT=WqA, rhs=h1augT[d], start=True, stop=True)
        nc.vector.tensor_copy(out=qT_sb[d], in_=qTp)
        kTp = psB.tile([128, S], FP32, tag="ps")
        nc.tensor.matmul(kTp, lhsT=WkA, rhs=h1augT[d], start=True, stop=True)
        nc.vector.tensor_copy(out=kT_sb[d], in_=kTp)

        vp = psB.tile([128, NCH, H * (DH + 1)], FP32, tag="ps")
        for c in range(NCH):
            nc.tensor.matmul(
                vp[:, c, :], lhsT=h1augT[d][:, 128 * c : 128 * (c + 1)], rhs=WvA,
                start=(c == 0), stop=(c == NCH - 1),
            )
        v_sb = pool2.tile([128, NCH, H * (DH + 1)], BF16, tag="v_sb")
        nc.vector.tensor_copy(out=v_sb, in_=vp)

        # attention per head
        pT = pool2.tile([128, NCH, S], BF16, tag="pT")
        ctxp = psC.tile([128, NCH, H * (DH + 1)], FP32, tag="ctxp")
        for h in range(H):
            q_h = qT_sb[d][32 * h : 32 * h + DH, :]
            for half in range(2):
                sc = psA.tile([128, 2, S], FP32, tag="scores")
                for j in range(2):
                    kc = 2 * half + j
                    k_h = kT_sb[d][32 * h : 32 * h + DH, 128 * kc : 128 * (kc + 1)]
                    nc.tensor.matmul(
                        sc[:, j, :], lhsT=k_h, rhs=q_h, start=True, stop=True,
                        tile_position=(32 * h, 0),
                    )
                nc.scalar.activation(
                    out=pT[:, 2 * half : 2 * half + 2, :], in_=sc,
                    func=mybir.ActivationFunctionType.Exp, scale=ISQD,
                )
            for kc in range(NCH):
                for qc in range(NCH):
                    nc.tensor.matmul(
                        ctxp[:, qc, 11 * h : 11 * h + 11],
                        lhsT=pT[:, kc, 128 * qc : 128 * (qc + 1)],
                        rhs=v_sb[:, kc, 11 * h : 11 * h + 11],
                        start=(h == 0 and kc == 0 and qc == 0),
                        stop=(h == H - 1 and kc == NCH - 1 and qc == NCH - 1),
                    )

        # normalize: ctxn[:, qc, 10h+j] = ctxu[:, qc, 11h+j] / denom[:, qc, h]
        recip = pool.tile([128, NCH, H], FP32, tag="recip")
        denom = bass.AP(
            ctxp.tensor, ctxp.offset + DH,
            [ctxp.ap[0], [H * (DH + 1), NCH], [DH + 1, H]],
        )
        nc.vector.reciprocal(out=recip, in_=denom)
        ctxn = pool2.tile([128, NCH, HID], BF16, tag="ctxn")
        ctxu_v = bass.AP(
            ctxp.tensor, ctxp.offset,
            [ctxp.ap[0], [H * (DH + 1), NCH], [DH + 1, H], [1, DH]],
        )
        ctxn_v = bass.AP(
            ctxn.tensor, ctxn.offset, [ctxn.ap[0], [HID, NCH], [DH, H], [1, DH]]
        )
        recip_v = bass.AP(
            recip.tensor, recip.offset, [recip.ap[0], [H, NCH], [1, H], [0, DH]]
        )
        nc.vector.tensor_tensor(
            out=ctxn_v, in0=ctxu_v, in1=recip_v, op=mybir.AluOpType.mult
        )

        # ctxn transpose -> Wo -> attn residual
        trC = psB.tile([HID, S], BF16, tag="ps")
        for c in range(NCH):
            nc.tensor.matmul(
                trC[:, 128 * c : 128 * (c + 1)], lhsT=ctxn[:, c, :], rhs=ident_bf,
                is_transpose=True, start=(c == 0), stop=(c == NCH - 1),
            )
        nc.vector.tensor_copy(out=ctxnaugT[d][0:HID, :], in_=trC)
        projp = psB.tile([128, NCH, HID], FP32, tag="ps")
        for c in range(NCH):
            nc.tensor.matmul(
                projp[:, c, :], lhsT=ctxnaugT[d][:, 128 * c : 128 * (c + 1)], rhs=WoA,
                start=(c == 0), stop=(c == NCH - 1),
            )
        nc.vector.tensor_tensor(
            out=attn_all[:, b, :, :], in0=projp, in1=x_all[:, b, :, :],
            op=mybir.AluOpType.add,
        )

        # LN2
        bn6b = pool.tile([128, NCH, 6], FP32, tag="bn6")
        mv2 = pool.tile([128, NCH, 2], FP32, tag="mv2")
        for c in range(NCH):
            nc.vector.bn_stats(out=bn6b[:, c, :], in_=attn_all[:, b, c, :])
            nc.vector.bn_aggr(out=mv2[:, c, :], in_=bn6b[:, c, :])
        var2 = bass.AP(mv2.tensor, mv2.offset + 1, [mv2.ap[0], [2, NCH]])
        lnv2 = pool.tile([128, NCH], FP32, tag="lnv2")
        nc.scalar.activation(
            out=lnv2, in_=var2, func=mybir.ActivationFunctionType.Ln, bias=EPS
        )
        rsig2 = pool.tile([128, NCH], FP32, tag="rsig2")
        nc.scalar.activation(
            out=rsig2, in_=lnv2, func=mybir.ActivationFunctionType.Exp, scale=-0.5
        )

        h2n = pool.tile([128, NCH, HID], BF16, tag="h2n")
        for c in range(NCH):
            nc.vector.tensor_scalar(
                out=h2n[:, c, :], in0=attn_all[:, b, c, :],
                scalar1=mv2[:, c, 0:1], scalar2=rsig2[:, c : c + 1],
                op0=mybir.AluOpType.subtract, op1=mybir.AluOpType.mult,
            )
        trH2 = psB.tile([HID, S], BF16, tag="ps")
        for c in range(NCH):
            nc.tensor.matmul(
                trH2[:, 128 * c : 128 * (c + 1)], lhsT=h2n[:, c, :], rhs=ident_bf,
                is_transpose=True, start=(c == 0), stop=(c == NCH - 1),
            )
        nc.vector.tensor_copy(out=h2augT[d][0:HID, :], in_=trH2)

        interTp = psB.tile([INTER, S], FP32, tag="ps")
        nc.tensor.matmul(interTp, lhsT=W1A, rhs=h2augT[d], start=True, stop=True)
        nc.vector.tensor_copy(out=interT_all[:, b, :], in_=interTp)

    # ---------------- phase 3: gelu (one table switch) + mm2 + out ----------------
    nc.scalar.activation(
        out=geluT_all[0:INTER, :, :], in_=interT_all[:, :, :],
        func=mybir.ActivationFunctionType.Gelu,
    )
    for b in range(bloc):
        m2p = psB.tile([128, NCH, HID], FP32, tag="ps")
        for c in range(NCH):
            nc.tensor.matmul(
                m2p[:, c, :], lhsT=geluT_all[:, b, 128 * c : 128 * (c + 1)], rhs=W2A,
                start=(c == 0), stop=(c == NCH - 1),
            )
        out_sb = pool.tile([128, NCH, HID], FP32, tag="out_sb")
        nc.vector.tensor_tensor(
            out=out_sb, in0=m2p, in1=attn_all[:, b, :, :], op=mybir.AluOpType.add
        )
        nc.sync.dma_start(
            out=out_dram[b].rearrange("(c p) d -> p c d", p=128), in_=out_sb
        )
    ctx.close()


_CACHE = {}


def _get_compiled(bloc=BLOC):
    key = bloc
    if key in _CACHE:
        return _CACHE[key]
    nc = bacc.Bacc("TRN2", target_bir_lowering=False, debug=False, num_devices=NCORES)
    ins = {}
    ins["hidden_states"] = nc.dram_tensor(
        "hidden_states", [bloc, S, HID], FP32, kind="ExternalInput"
    ).ap()
    for name, shape in PARAMS:
        ins[name] = nc.dram_tensor(name, list(shape), FP32, kind="ExternalInput").ap()
    out_dram = nc.dram_tensor("out", [bloc, S, HID], FP32, kind="ExternalOutput").ap()
    with tile.TileContext(nc) as tc:
        build_bert_kernel(nc, tc, ins, out_dram, bloc=bloc)
    nc.compile()
    _CACHE[key] = (nc, out_dram)
    return _CACHE[key]


def kernel(**inputs):
    from concourse.bass_utils import run_bass_kernel_spmd

    nc, out_dram = _get_compiled()
    x = np.ascontiguousarray(np.asarray(inputs["hidden_states"], dtype=np.float32))
    params = {
        name: np.ascontiguousarray(np.asarray(inputs[name], dtype=np.float32))
        for name, _ in PARAMS
    }
    in_maps = []
    for core in range(NCORES):
        m = dict(params)
        m["hidden_states"] = x[core * BLOC : (core + 1) * BLOC]
        in_maps.append(m)
    res = run_bass_kernel_spmd(nc, in_maps, core_ids=list(range(NCORES)))
    out = np.concatenate([res.results[i]["out"] for i in range(NCORES)], axis=0)
    return out.astype(np.float32)
